# revision 1
# baseline (speedup 1.0000x reference)
"""Trainium2 Bass kernel for nn_Caps_36215164240532.

Computes, per batch element b (inputs x of shape (B, 2, 96)):
  qkv/BN1 -> 3-head attention over the 2 rows -> BN2 -> linear MLP -> BN3
  -> LSTM-style gate combine -> next_memory, output = next_memory duplicated
  on axis 1.

Key algebraic reductions (done on host, exact):
  * All BatchNorms are inference-affine -> folded into matmul weights/biases.
  * The MLP is linear -> (nm@W1+b1)@W2+b2 collapses with BN2/BN3 and the
    residual into a single 96x96 matmul M plus bias.
  * Only row 0 of the post-attention path is used downstream (nm1 = nm3[:,0]),
    so row-1 attention output / MLP are dead code.
  * 2-way softmax == sigmoid of the score difference:
      att0 = v1 + sigmoid(q0.(k0-k1)) * (v0-v1)
    and k0-k1 = (X0-X1)@Wk, v0-v1 = (X0-X1)@Wv (projection is linear).
  * nm3 = X0@Ma + X1@Mb + (w*vd)@Mc + cvec ; out = sig_i*tanh(nm3) + sig_f*X0

Data-parallel over 8 NeuronCores (batch sharded); weights replicated.
On-chip layout is feature-major ([96 features on partitions, elements on the
free dim]); transposes to/from the DRAM element-major layout run on the
tensor engine against identity matrices.
"""

import numpy as np

import concourse.mybir as mybir
import concourse.tile as tile
from concourse import bacc
from concourse.bass_utils import run_bass_kernel_spmd

F32 = mybir.dt.float32
F32R = mybir.dt.float32r
AF = mybir.ActivationFunctionType
ALU = mybir.AluOpType


N_CORES = 8
B_FULL = 131072
D = 96
PER = B_FULL // N_CORES        # 16384 elements per core
CHUNK = 512                    # elements per compute chunk
NCHUNK = PER // CHUNK          # 32
GROUP_CHUNKS = 4               # chunks per DMA group
GROUP = CHUNK * GROUP_CHUNKS   # 2048 elements per DMA transfer
NGROUP = PER // GROUP          # 8
NSUB = GROUP // 128            # 16 sub-chunks of 128 elements per group
EPS = 1e-3
ILEAVE = 5                     # chunks interleaved for pipelining

def _fold_weights(w):
    """Fold BN/bias/MLP algebra into the minimal constant set (float64)."""
    f64 = lambda x: np.asarray(x, np.float64)
    Wqkv = f64(w["Wqkv"])
    s1 = 1.0 / np.sqrt(f64(w["bn1_v"]) + EPS) * f64(w["bn1_g"])
    Wqkv_f = Wqkv * s1[None, :]
    bqkv_f = (f64(w["bqkv"]) - f64(w["bn1_m"])) * s1 + f64(w["bn1_b"])

    idx_q = np.concatenate([np.arange(h * 96, h * 96 + 32) for h in range(3)])
    Wq, bq = Wqkv_f[:, idx_q], bqkv_f[idx_q]
    Wk = Wqkv_f[:, idx_q + 32]
    Wv, bv = Wqkv_f[:, idx_q + 64], bqkv_f[idx_q + 64]

    s2 = 1.0 / np.sqrt(f64(w["bn2_v"]) + EPS) * f64(w["bn2_g"])
    beta2 = f64(w["bn2_b"]) - f64(w["bn2_m"]) * s2
    s3 = 1.0 / np.sqrt(f64(w["bn3_v"]) + EPS) * f64(w["bn3_g"])
    beta3 = f64(w["bn3_b"]) - f64(w["bn3_m"]) * s3

    W12 = f64(w["W1"]) @ f64(w["W2"])
    b12 = f64(w["b1"]) @ f64(w["W2"]) + f64(w["b2"])
    G = (W12 + np.eye(D)) * s3[None, :]
    M = s2[:, None] * G
    c = beta2 @ G + b12 * s3 + beta3

    consts = {
        "wq": Wq, "wk": Wk, "wv": Wv,
        "ma": M, "mb": Wv @ M, "mc": M,
        "bq": bq.reshape(D, 1),
        "cvec": (c + bv @ M).reshape(D, 1),
    }
    gb = f64(w["bgi"]) + f64(w["bgm"]) + np.array([0.0, 1.0])

    # gate stationaries: ig -> psum row 0, fg -> psum row 32 (zero-padded so
    # all 34 written rows are initialized; fp32r matmuls must write base 0).
    # wgm has a 97th row carrying the gate bias, matched by tm's ones-row.
    wg_i = np.zeros((D, 34))
    wg_m = np.zeros((D, 34))
    wg_i[:, 0], wg_m[:, 0] = f64(w["Wgi"])[:, 0], f64(w["Wgm"])[:, 0]
    wg_i[:, 32], wg_m[:, 32] = f64(w["Wgi"])[:, 1], f64(w["Wgm"])[:, 1]
    consts["wgi"] = wg_i
    consts["wgm"] = wg_m
    sgb = np.zeros((34, 1))
    sgb[0, 0] = gb[0]
    sgb[32, 0] = gb[1]
    consts["sgbias"] = sgb

    onesb = np.zeros((D, 34))          # per-head column sums, rows 0..33
    for h in range(3):
        onesb[h * 32:(h + 1) * 32, h] = 1.0
    consts["onesb"] = onesb

    cb = np.zeros((33, D))             # broadcast rows: ig row 0, fg row 32
    cb[0, :] = 1.0
    cb[32, :] = 1.0
    consts["cb"] = cb

    onese = np.zeros((3, D))           # per-head broadcast of sigmoid weights
    for h in range(3):
        onese[h, h * 32:(h + 1) * 32] = 1.0
    consts["onese"] = onese



    consts["ident"] = np.eye(128)
    return {k: np.asarray(v, np.float32) for k, v in consts.items()}


_CONST_SHAPES = {
    "wq": (D, D), "wk": (D, D), "wv": (D, D),
    "ma": (D, D), "mb": (D, D), "mc": (D, D),
    "bq": (D, 1), "cvec": (D, 1),
    "wgi": (D, 34), "wgm": (D, 34),
    "onesb": (D, 34), "cb": (33, D), "onese": (3, D),
    "sgbias": (34, 1), "ident": (128, 128),
}

# matmul-operand dtype: float32r streams 4x faster through the PE than fp32
MMDT = F32R
# constants that participate in fp32r matmuls (stationary operands)
_MM_CONSTS = {"wq", "wk", "wv", "ma", "mb", "mc", "wgi", "wgm",
              "onesb", "cb", "onese", "ident"}


def _cdt(name):
    return MMDT if name in _MM_CONSTS else F32


def _build_program(per=PER, chunk=CHUNK, group_chunks=GROUP_CHUNKS, debug=False,
                   lrepeat=1):
    CHUNK = chunk
    GROUP_CHUNKS = group_chunks
    GROUP = CHUNK * GROUP_CHUNKS
    NGROUP = per // GROUP
    NSUB = GROUP // 128
    CSUB = CHUNK // 128          # 128-element sub-chunks per chunk

    ILV = min(ILEAVE, GROUP_CHUNKS)
    nc = bacc.Bacc("TRN2", target_bir_lowering=False, debug=debug)
    x_dram = nc.dram_tensor("x", [per, 2 * D], F32R, kind="ExternalInput").ap()
    out_dram = nc.dram_tensor("out", [per, D], F32, kind="ExternalOutput").ap()
    const_dram = {
        name: nc.dram_tensor(name, list(shape), _cdt(name),
                             kind="ExternalInput").ap()
        for name, shape in _CONST_SHAPES.items()
    }

    xr = x_dram.rearrange("(g s p) f -> g p s f", p=128, s=NSUB)
    orr = out_dram.rearrange("(g s p) f -> g p s f", p=128, s=NSUB)

    with tile.TileContext(nc) as tc:
        with (
            tc.tile_pool(name="const", bufs=1) as cpool,
            tc.tile_pool(name="io", bufs=3) as iopool,
            tc.tile_pool(name="sb", bufs=5) as sb,
            tc.tile_pool(name="pss", bufs=8, space="PSUM") as pss_pool,
        ):
            C = {}
            for name, shape in _CONST_SHAPES.items():
                t = cpool.tile(list(shape), _cdt(name), tag=name)
                nc.sync.dma_start(t[:], const_dram[name][:])
                C[name] = t

            def chunk_ops(j, x_nat, o_nat):
                """Generator emitting one chunk's ops; yields between stages
                so two chunks can be interleaved for pipelining."""
                ps_x0 = pss_pool.tile([D, CHUNK], F32R, tag="small")
                ps_x1 = pss_pool.tile([D, CHUNK], F32R, tag="small")
                for s in range(CSUB):
                    off = (j * CSUB + s) * 2 * D
                    nc.tensor.transpose(
                        ps_x0[:, s * 128:(s + 1) * 128],
                        x_nat[:, off:off + D], C["ident"][:])
                    nc.tensor.transpose(
                        ps_x1[:, s * 128:(s + 1) * 128],
                        x_nat[:, off + D:off + 2 * D], C["ident"][:])
                yield
                x_s = sb.tile([D, 2 * CHUNK], MMDT, tag="x_s")
                nc.scalar.copy(x_s[:, 0:CHUNK], ps_x0[:].bitcast(F32))
                nc.scalar.copy(x_s[:, CHUNK:], ps_x1[:].bitcast(F32))
                tm = sb.tile([D, CHUNK], MMDT, tag="tm")
                nc.scalar.activation(tm[:], ps_x0[:].bitcast(F32), AF.Tanh)
                yield
                xd = sb.tile([D, CHUNK], MMDT, tag="xd")
                nc.gpsimd.tensor_sub(xd[:], x_s[:, 0:CHUNK].bitcast(F32),
                                     x_s[:, CHUNK:].bitcast(F32))
                yield
                ps_kd = pss_pool.tile([D, CHUNK], F32, tag="small")
                ps_vd = pss_pool.tile([D, CHUNK], F32, tag="small")
                ps_q = pss_pool.tile([D, CHUNK], F32, tag="small")
                nc.tensor.matmul(ps_kd[:], C["wk"][:], xd[:])
                nc.tensor.matmul(ps_vd[:], C["wv"][:], xd[:])
                nc.tensor.matmul(ps_q[:], C["wq"][:], x_s[:, 0:CHUNK])
                yield
                kv_s = sb.tile([D, 2 * CHUNK], F32, tag="kv_s")
                nc.vector.tensor_copy(kv_s[:, 0:CHUNK], ps_kd[:])
                nc.vector.tensor_copy(kv_s[:, CHUNK:], ps_vd[:])
                yield
                p0 = sb.tile([D, CHUNK], MMDT, tag="p0")
                nc.vector.scalar_tensor_tensor(
                    p0[:], ps_q[:], C["bq"][:], kv_s[:, 0:CHUNK],
                    ALU.add, ALU.mult)
                yield
                ps_d = pss_pool.tile([34, CHUNK], F32, tag="small")
                nc.tensor.matmul(ps_d[:], C["onesb"][:], p0[:])
                ps_g = pss_pool.tile([34, CHUNK], F32, tag="small")
                nc.tensor.matmul(ps_g[:], C["wgi"][:],
                                 x_s[:, CHUNK:], start=True, stop=False)
                nc.tensor.matmul(ps_g[:], C["wgm"][:],
                                 tm[:], start=False, stop=True)
                yield
                sg = sb.tile([34, 2 * CHUNK], MMDT, tag="sg")
                nc.scalar.activation(sg[:, 0:CHUNK], ps_d[:], AF.Sigmoid)
                nc.scalar.activation(sg[:, CHUNK:], ps_g[:], AF.Sigmoid,
                                     bias=C["sgbias"][:])
                yield
                ps_w = pss_pool.tile([D, CHUNK], F32, tag="small")
                nc.tensor.matmul(ps_w[:], C["onese"][:], sg[0:3, 0:CHUNK])
                ps_ib = pss_pool.tile([D, CHUNK], F32, tag="small")
                ps_fb = pss_pool.tile([D, CHUNK], F32, tag="small")
                nc.tensor.matmul(ps_ib[:], C["cb"][0:1, :], sg[0:1, CHUNK:])
                nc.tensor.matmul(ps_fb[:], C["cb"][32:33, :],
                                 sg[32:33, CHUNK:])
                yield
                wvd = sb.tile([D, CHUNK], MMDT, tag="wvd")
                nc.vector.tensor_mul(wvd[:], ps_w[:], kv_s[:, CHUNK:])
                yield
                ps_nm = pss_pool.tile([D, CHUNK], F32, tag="small")
                nc.tensor.matmul(ps_nm[:], C["ma"][:], x_s[:, 0:CHUNK],
                                 start=True, stop=False)
                nc.tensor.matmul(ps_nm[:], C["mb"][:], x_s[:, CHUNK:],
                                 start=False, stop=False)
                nc.tensor.matmul(ps_nm[:], C["mc"][:], wvd[:],
                                 start=False, stop=True)
                yield
                t_s = sb.tile([D, CHUNK], F32, tag="t_s")
                nc.scalar.activation(t_s[:], ps_nm[:], AF.Tanh,
                                     bias=C["cvec"][:])
                yield
                s1 = sb.tile([D, CHUNK], F32, tag="s1")
                nc.vector.tensor_mul(s1[:], ps_ib[:], t_s[:])
                u = sb.tile([D, CHUNK], F32, tag="u")
                nc.vector.tensor_mul(u[:], ps_fb[:],
                                     x_s[:, 0:CHUNK].bitcast(F32))
                yield
                nxt = sb.tile([D, CHUNK], F32R, tag="nxt")
                nc.vector.tensor_add(nxt[:], s1[:], u[:])
                yield
                ps_o = pss_pool.tile([128, CSUB * D], F32R, tag="small")
                for s in range(CSUB):
                    nc.tensor.transpose(
                        ps_o[:, s * D:(s + 1) * D],
                        nxt[:, s * 128:(s + 1) * 128],
                        C["ident"][0:D, 0:D])
                yield
                nc.scalar.copy(o_nat[:, j * CSUB * D:(j + 1) * CSUB * D],
                               ps_o[:].bitcast(F32))

            # sliding-window interleave across ALL chunks (and group
            # boundaries): ILV chunk-generators advance round-robin; group
            # input DMAs are emitted lazily, output DMAs when a group's last
            # chunk retires.  io pool bufs bound the groups in flight.
            from collections import deque

            instances = [(gi, g, j)
                         for gi, g in enumerate(
                             [g for _ in range(lrepeat)
                              for g in range(NGROUP)])
                         for j in range(GROUP_CHUNKS)]
            group_res = {}

            def ensure_group(gi, g):
                if gi not in group_res:
                    x_nat = iopool.tile([128, NSUB * 2 * D], F32R, tag="x_nat")
                    nc.sync.dma_start(
                        x_nat[:].rearrange("p (s f) -> p s f", s=NSUB), xr[g])
                    o_nat = iopool.tile([128, NSUB * D], F32, tag="o_nat")
                    group_res[gi] = [x_nat, o_nat, 0, g]
                return group_res[gi]

            def retire_chunk(gi):
                res = group_res[gi]
                res[2] += 1
                if res[2] == GROUP_CHUNKS:
                    nc.sync.dma_start(
                        orr[res[3]],
                        res[1][:].rearrange("p (s f) -> p s f", s=NSUB))
                    del group_res[gi]

            window = deque()
            it = iter(instances)
            pending = True
            while window or pending:
                while pending and len(window) < ILV:
                    try:
                        gi, g, j = next(it)
                    except StopIteration:
                        pending = False
                        break
                    res = ensure_group(gi, g)
                    window.append((gi, chunk_ops(j, res[0], res[1])))
                for _ in range(len(window)):
                    gi, gen = window.popleft()
                    try:
                        next(gen)
                        window.append((gi, gen))
                    except StopIteration:
                        retire_chunk(gi)

    nc.compile()
    return nc


_prog_cache = {}


def _get_program():
    if "nc" not in _prog_cache:
        _prog_cache["nc"] = _build_program()
    return _prog_cache["nc"]


def _run(inputs, trace=False):
    x = np.ascontiguousarray(
        np.asarray(inputs["inputs"], np.float32).reshape(B_FULL, 2 * D)
    )
    consts = _fold_weights(inputs)
    nc = _get_program()
    in_maps = []
    for i in range(N_CORES):
        m = {"x": x[i * PER:(i + 1) * PER]}
        m.update(consts)
        in_maps.append(m)
    try:
        res = run_bass_kernel_spmd(nc, in_maps, list(range(N_CORES)),
                                   trace=trace)
    except Exception:
        res = run_bass_kernel_spmd(nc, in_maps, list(range(N_CORES)),
                                   trace=trace)
    rows = np.concatenate([res.results[i]["out"] for i in range(N_CORES)], axis=0)
    full = np.repeat(rows.reshape(B_FULL, 1, D), 2, axis=1)
    return full, res


def kernel(**inputs) -> np.ndarray:
    out, _ = _run(inputs, trace=False)
    return out

